# revision 6
# baseline (speedup 1.0000x reference)
"""YOLOv1 loss (nn_LossModul_16277926052544) on 8 TRN2 NeuronCores.

Pure data parallel: batch 8192 -> 8 shards of 1024 (= 50176 grid cells/core,
128 partitions x F=392). Each core computes a partial loss; host sums.

v2 design (vs 75us baseline):
  * All box channels bf16, cls channels fp8-e3m4 in DRAM (numpy-validated
    rel err 7.0e-4 vs the 2e-2 gate). HBM reads drop 11.3MB -> 3.9MB/core.
  * cls difference (pc - tc) computed by the DMA engine: pcls is cast
    fp8->bf16 in flight (SWDGE), then -tcls lands on the same tile with
    accum_op=add. Removes a 20-channel DVE pass entirely.
  * Geometry runs once at F=392 (per-instr 58cyc bubble halved vs T=2);
    cls mask/square split in two F-halves for DMA overlap.
  * Select block: one contiguous 5ch copy (ACT) + one predicated copy,
    enabled by IoU translation invariance (grid offsets cancel; overlap
    len per axis = min(max((pw+tw)/2 - |dx|/S, 0), pw, tw)).
  * resp = iou1>iou2 <=> I1*D2 > I2*D1; iou_sel via custom-DVE fast
    reciprocal. All loss terms fold their lambda into per-cell masks so
    three ACT Square+accum instructions reduce everything.
"""
import sys

for _p in ("/opt/trn_rl_repo",):
    if _p not in sys.path:
        sys.path.insert(0, _p)

import numpy as np
import ml_dtypes
from contextlib import ExitStack

import concourse.bass as bass  # noqa: F401  (registers engines)
from concourse import bacc, mybir
from concourse import bass_utils
import concourse.tile as tile

N_CORES = 8
BATCH = 8192
S = 7
P = 128
CELLS = (BATCH // N_CORES) * S * S            # 50176
FF = CELLS // P                               # 392
FH = FF // 2                                  # 196
R = 1.0 / S
EPS5 = 5e-6                                   # 5 * EPS (lambda folded)
SQRT5 = float(np.sqrt(5.0))
SQH = float(np.sqrt(0.5))

f32 = mybir.dt.float32
bf16 = mybir.dt.bfloat16
fp8 = mybir.dt.float8e3                       # e3m4
Alu = mybir.AluOpType
Act = mybir.ActivationFunctionType

_CACHE = {}


def _build_body(tc, ctx, pb_d, tb_d, pc_d, tn_d, out_ap):
    nc = tc.nc
    pool = ctx.enter_context(tc.tile_pool(name="w", bufs=1))
    t = lambda shape, dt, tag: pool.tile(shape, dt, tag=tag, name=tag)

    eps5c = t([P, 1], f32, "eps5c")
    nc.gpsimd.memset(eps5c[:], EPS5)

    pb = t([P, 10, FF], bf16, "pb")       # [x1,y1,w1,h1,c1,x2,y2,w2,h2,c2]
    tb = t([P, 5, FF], bf16, "tb")        # [tx,ty,tw,th,tconf]
    dt_ = t([P, 27, FF], bf16, "dt")      # 0:20 cls diff, 20:27 sel slots
    stats = t([P, 3], f32, "stats")

    # ---- DMAs: box via HWDGE; cls via SWDGE with fp8->bf16 cast, then
    # -tcls accumulated on top (d = pc - tc materializes on arrival).
    nc.sync.dma_start(tb[:], tb_d)
    nc.sync.dma_start(pb[:], pb_d)
    for h in range(2):
        fs = slice(h * FH, (h + 1) * FH)
        nc.gpsimd.dma_start(dt_[:, 0:20, fs], pc_d[:, :, fs])
        nc.gpsimd.dma_start(dt_[:, 0:20, fs], tn_d[:, :, fs],
                            accum_op=Alu.add)

    # ---- masks (lambdas folded in) ----
    M = t([P, 4, FF], bf16, "M")          # mo, sqrt5*mo, 5*mo, sqrt(.5)(1-mo)
    nc.vector.tensor_single_scalar(M[:, 0], tb[:, 4], 0.0, op=Alu.is_gt)
    nc.vector.tensor_scalar_mul(M[:, 1], M[:, 0], SQRT5)
    nc.vector.tensor_scalar_mul(M[:, 2], M[:, 0], 5.0)
    nc.vector.tensor_scalar(M[:, 3], M[:, 0], -SQH, SQH,
                            op0=Alu.mult, op1=Alu.add)
    mo = M[:, 0:1]                         # [P,1,FF] for broadcasts

    # ---- geometry, both boxes as [P, 2, 2*FF] (inner = (x|w ch, y|h ch)) --
    pbv = pb[:].rearrange("p (b c) f -> p b (c f)", b=2)   # [P,2,5*FF]
    p_xy = pbv[:, :, 0:2 * FF]
    p_wh = pbv[:, :, 2 * FF:4 * FF]
    tbf = tb[:].rearrange("p c f -> p (c f)")
    t_xy = tbf[:, 0:2 * FF].unsqueeze(1).broadcast_to([P, 2, 2 * FF])
    t_wh = tbf[:, 2 * FF:4 * FF].unsqueeze(1).broadcast_to([P, 2, 2 * FF])

    sth = t([P, 2 * FF], bf16, "sth")     # 0.5*twh
    nc.vector.tensor_scalar_mul(sth[:], tbf[:, 2 * FF:4 * FF], 0.5)
    sth_b = sth[:].unsqueeze(1).broadcast_to([P, 2, 2 * FF])

    # dxy lands in-place over pb's xy channels (so the contiguous 5ch
    # select below picks up [dx,dy,w,h,c] directly)
    nc.vector.tensor_sub(p_xy, p_xy, t_xy)
    absd = t([P, 2, 2 * FF], bf16, "absd")
    nc.vector.scalar_tensor_tensor(absd[:], p_xy, -1.0, p_xy,
                                   op0=Alu.mult, op1=Alu.max)
    sm = t([P, 2, 2 * FF], bf16, "sm")    # (pw+tw)/2
    nc.vector.scalar_tensor_tensor(sm[:], p_wh, 0.5, sth_b,
                                   op0=Alu.mult, op1=Alu.add)
    mm = t([P, 2, 2 * FF], bf16, "mm")    # s - R|d|
    nc.vector.scalar_tensor_tensor(mm[:], absd[:], -R, sm[:],
                                   op0=Alu.mult, op1=Alu.add)
    t1 = t([P, 2, 2 * FF], bf16, "t1")    # min(max(m,0), pw)
    nc.vector.scalar_tensor_tensor(t1[:], mm[:], 0.0, p_wh,
                                   op0=Alu.max, op1=Alu.min)
    ln = t([P, 2, 2 * FF], bf16, "ln")    # overlap lengths
    nc.vector.tensor_tensor(ln[:], t1[:], t_wh, op=Alu.min)

    ID = t([P, 4, FF], bf16, "ID")        # [I1,I2,D1,D2]
    ln4 = ln[:].rearrange("p b (a f) -> p b a f", a=2)
    nc.vector.tensor_mul(ID[:, 0:2], ln4[:, :, 0], ln4[:, :, 1])
    ap_ = t([P, 2, FF], bf16, "ap")
    nc.vector.tensor_mul(ap_[:], pb[:, 2:8:5], pb[:, 3:9:5])
    at_ = t([P, 1, FF], bf16, "at")
    nc.vector.tensor_mul(at_[:], tb[:, 2:3], tb[:, 3:4])
    nc.vector.tensor_sub(ap_[:], ap_[:], ID[:, 0:2])
    nc.vector.tensor_add(ID[:, 2:4], ap_[:], at_[:].broadcast_to([P, 2, FF]))

    g = t([P, 2, FF], bf16, "g")
    nc.vector.tensor_mul(g[:, 0], ID[:, 0], ID[:, 3])
    nc.vector.tensor_mul(g[:, 1], ID[:, 1], ID[:, 2])
    resp = t([P, 1, FF], mybir.dt.uint16, "resp")  # 1 -> box1 responsible
    nc.vector.tensor_tensor(resp[:, 0], g[:, 0], g[:, 1], op=Alu.is_gt)

    # ---- selects: box2 copied, box1 predicated over it ----
    sel = dt_[:, 20:27]                   # [dxs,dys,ws,hs,cs,n1,n2]
    nc.scalar.copy(sel[:, 0:5], pb[:, 5:10])
    nc.vector.copy_predicated(sel[:, 0:5],
                              resp[:].broadcast_to([P, 5, FF]), pb[:, 0:5])
    idsel = t([P, 2, FF], bf16, "idsel")  # [Isel, Dsel]
    nc.vector.tensor_copy(idsel[:], ID[:, 1:4:2])
    nc.vector.copy_predicated(idsel[:], resp[:].broadcast_to([P, 2, FF]),
                              ID[:, 0:3:2])

    dful = t([P, 1, FF], f32, "dful")
    nc.vector.tensor_copy(dful[:], idsel[:, 1:2])
    rcp = t([P, 1, FF], f32, "rcp")
    nc.vector.reciprocal_approx_fast(rcp[:, 0], dful[:, 0])
    iou = t([P, 1, FF], f32, "iou")
    nc.vector.tensor_mul(iou[:, 0], idsel[:, 0], rcp[:, 0])
    nc.vector.scalar_tensor_tensor(sel[:, 4], iou[:, 0], -1.0, sel[:, 4],
                                   op0=Alu.mult, op1=Alu.add)
    nc.vector.tensor_mul(sel[:, 4], sel[:, 4], M[:, 0])

    # ---- masked slots ----
    nc.vector.tensor_mul(sel[:, 0:2], sel[:, 0:2],
                         M[:, 1:2].broadcast_to([P, 2, FF]))
    uv = t([P, 4, FF], bf16, "uv")        # [5mo*ws, 5mo*hs, 5mo*tw, 5mo*th]
    nc.vector.tensor_mul(uv[:, 0:2], sel[:, 2:4],
                         M[:, 2:3].broadcast_to([P, 2, FF]))
    nc.vector.tensor_mul(uv[:, 2:4], tb[:, 2:4],
                         M[:, 2:3].broadcast_to([P, 2, FF]))
    w_ = t([P, 4, FF], bf16, "w")
    nc.scalar.activation(w_[:], uv[:], Act.Sqrt, bias=eps5c[:])
    nc.vector.tensor_sub(sel[:, 2:4], w_[:, 0:2], w_[:, 2:4])
    nc.vector.tensor_mul(sel[:, 5:7], pb[:, 4:10:5],
                         M[:, 3:4].broadcast_to([P, 2, FF]))

    # ---- cls mask + square-accumulate ----
    nc.scalar.activation(sel[:], sel[:], Act.Square,
                         accum_out=stats[:, 0:1])
    for h in range(2):
        fs = slice(h * FH, (h + 1) * FH)
        nc.vector.tensor_mul(dt_[:, 0:20, fs], dt_[:, 0:20, fs],
                             mo[:, :, fs].broadcast_to([P, 20, FH]))
        nc.scalar.activation(dt_[:, 0:20, fs], dt_[:, 0:20, fs], Act.Square,
                             accum_out=stats[:, 1 + h:2 + h])

    total = t([P, 1], f32, "total")
    nc.vector.tensor_reduce(total[:], stats[:], axis=mybir.AxisListType.X,
                            op=Alu.add)
    nc.sync.dma_start(out_ap, total[:])


def _build():
    if "nc" in _CACHE:
        return _CACHE["nc"]
    nc = bacc.Bacc("TRN2", target_bir_lowering=False, debug=False)
    pb_d = nc.dram_tensor("pbox", [P, 10, FF], bf16, kind="ExternalInput")
    tb_d = nc.dram_tensor("tbox", [P, 5, FF], bf16, kind="ExternalInput")
    pc_d = nc.dram_tensor("pcls", [P, 20, FF], fp8, kind="ExternalInput")
    tn_d = nc.dram_tensor("tclsn", [P, 20, FF], fp8, kind="ExternalInput")
    out = nc.dram_tensor("out", [P, 1], f32, kind="ExternalOutput")
    with tile.TileContext(nc) as tc, ExitStack() as ctx:
        _build_body(tc, ctx, pb_d.ap(), tb_d.ap(), pc_d.ap(), tn_d.ap(),
                    out.ap())
    nc.compile()
    _CACHE["nc"] = nc
    return nc


def _shard(predicts, targets):
    p = np.ascontiguousarray(predicts, dtype=np.float32)
    tg = np.ascontiguousarray(targets, dtype=np.float32)
    n = BATCH // N_CORES
    maps = []
    for i in range(N_CORES):
        ps = p[i * n:(i + 1) * n].reshape(P, FF, 30).transpose(0, 2, 1)
        ts = tg[i * n:(i + 1) * n].reshape(P, FF, 30).transpose(0, 2, 1)
        maps.append({
            "pbox": np.ascontiguousarray(ps[:, 0:10]).astype(
                ml_dtypes.bfloat16),
            "tbox": np.ascontiguousarray(ts[:, 0:5]).astype(
                ml_dtypes.bfloat16),
            "pcls": np.ascontiguousarray(ps[:, 10:30]).astype(
                ml_dtypes.float8_e3m4),
            "tclsn": np.ascontiguousarray(-ts[:, 10:30]).astype(
                ml_dtypes.float8_e3m4),
        })
    return maps


def run(predicts, targets, trace=False, **trace_kwargs):
    nc = _build()
    in_maps = _shard(predicts, targets)
    res = bass_utils.run_bass_kernel_spmd(
        nc, in_maps, core_ids=list(range(N_CORES)), trace=trace,
        **trace_kwargs)
    partial = np.zeros((), dtype=np.float64)
    for r in res.results:
        partial += np.asarray(r["out"], dtype=np.float64).sum()
    return np.float32(partial), res


def kernel(predicts, targets):
    out, _ = run(predicts, targets, trace=False)
    return out
